# revision 13
# baseline (speedup 1.0000x reference)
"""Expert-parallel MoE SwiGLU kernel for 8 Trainium2 NeuronCores.

Strategy: expert parallelism with host-side dispatch/combine. Each of the
8 cores owns one expert's weights. The host routes tokens by expert_idx,
packs each expert's tokens as a transposed [D, W] panel (features on
partitions so no on-chip transposes are needed anywhere), and each core
runs a dense SwiGLU FFN:  yT = w_down.T-blocks @ (silu(wg.T@xT) * (wu.T@xT)).
Matmuls run as float32r (full-rate fp32) with fp32 PSUM accumulation.
"""

import numpy as np
from contextlib import ExitStack

D_MODEL = 1024
D_FF = 4096
N_EXPERTS = 8
N_CORES = 8

_ND = D_MODEL // 128  # 8 contraction chunks over d_model
_NF = D_FF // 128     # 32 f chunks

_nc_cache = {}

# compute dtype for matmul operands: "float32r" (safest), "float16", "bfloat16"
import os as _os
_CDT = _os.environ.get("MOE_KERNEL_DTYPE", "float16")

# f columns per gate/up weight streaming group: keep DMA lines at 2KB
_FSG = 512 if _CDT == "float32r" else 1024
_NFSG = D_FF // _FSG
_FTG = _FSG // 128    # f-tiles per group


def _np_cdt():
    if _CDT == "float16":
        return np.float16
    if _CDT == "bfloat16":
        import ml_dtypes
        return ml_dtypes.bfloat16
    return np.float32


def _build_nc(W: int):
    """Build + schedule the per-core Bass program for token capacity W."""
    import concourse.bacc as bacc
    import concourse.tile as tile
    from concourse import mybir

    f32 = mybir.dt.float32
    f32r = getattr(mybir.dt, _CDT)

    nc = bacc.Bacc("TRN2", target_bir_lowering=False, debug=False,
                   num_devices=N_CORES)
    xt = nc.dram_tensor("xt", [D_MODEL, W], f32r, kind="ExternalInput").ap()
    wg = nc.dram_tensor("wg", [_NFSG, _ND, 128, _FSG], f32r,
                        kind="ExternalInput").ap()
    wu = nc.dram_tensor("wu", [_NFSG, _ND, 128, _FSG], f32r,
                        kind="ExternalInput").ap()
    wd = nc.dram_tensor("wd", [D_FF, D_MODEL], f32r, kind="ExternalInput").ap()
    yt = nc.dram_tensor("yt", [D_MODEL, W], f32, kind="ExternalOutput").ap()

    with tile.TileContext(nc) as tc, ExitStack() as ctx:
        xpool = ctx.enter_context(tc.tile_pool(name="x", bufs=1))
        wgp = ctx.enter_context(tc.tile_pool(name="wgp", bufs=3))
        wup = ctx.enter_context(tc.tile_pool(name="wup", bufs=3))
        wdp = ctx.enter_context(tc.tile_pool(name="wdp", bufs=2))
        tp = ctx.enter_context(tc.tile_pool(name="tp", bufs=2))
        gap = ctx.enter_context(tc.tile_pool(name="gap", bufs=3))
        yp = ctx.enter_context(tc.tile_pool(name="yp", bufs=1))
        pg = ctx.enter_context(tc.tile_pool(name="pg", bufs=2, space="PSUM"))
        pu = ctx.enter_context(tc.tile_pool(name="pu", bufs=2, space="PSUM"))
        pd = ctx.enter_context(tc.tile_pool(name="pd", bufs=4, space="PSUM"))

        # Input activations, transposed: d_model on partitions. SWDGE ring
        # so they don't delay the first weight tiles on the sync ring.
        xts = []
        for d in range(_ND):
            x_t = xpool.tile([128, W], f32r, tag=f"x{d}")
            nc.gpsimd.dma_start(x_t[:], xt[d * 128:(d + 1) * 128, :])
            xts.append(x_t)

        # f-group 0 gate/up weights arrive as quarter-width tiles so the
        # first matmul groups are fed after ~0.5MB instead of the full 2MB
        # (whole-tile dependency granularity would otherwise stall the PE
        # through the DMA ramp-up).
        _NQ = 4
        qw = _FSG // _NQ
        w0p = ctx.enter_context(tc.tile_pool(name="w0", bufs=1))
        w0 = {}
        for q in range(_NQ):
            for d in range(_ND):
                g0 = w0p.tile([128, qw], f32r, tag=f"g0_{d}_{q}",
                              name=f"g0_{d}_{q}")
                nc.sync.dma_start(g0[:], wg[0, d, :, q * qw:(q + 1) * qw])
                w0[("g", d, q)] = g0
        for q in range(_NQ):
            for d in range(_ND):
                u0 = w0p.tile([128, qw], f32r, tag=f"u0_{d}_{q}",
                              name=f"u0_{d}_{q}")
                nc.sync.dma_start(u0[:], wu[0, d, :, q * qw:(q + 1) * qw])
                w0[("u", d, q)] = u0

        y_acc = [yp.tile([128, W], f32, tag=f"y{d}", name=f"y_acc{d}")
                 for d in range(_ND)]

        # Fused pipeline over f groups: gate/up matmuls + SwiGLU produce
        # short-lived t tiles; the down-projection of the PREVIOUS f group
        # is interleaved between this group's matmul bursts so the PE's
        # DMA-wait gaps are broken into sub-HAM-window slices. Weight DMA
        # issue is split across both HWDGE rings (sync + scalar engines).
        def emit_down(fsg, t_tiles, wd_tiles, dts):
            # y[dt] += wd[fgroup rows, dt cols].T @ t   for dt in dts
            for dt in dts:
                pdt = pd.tile([128, W], f32, tag="pd", name=f"pd_{fsg}_{dt}")
                for ft in range(_FTG):
                    nc.tensor.matmul(
                        pdt[:],
                        wd_tiles[ft][:, dt * 128:(dt + 1) * 128],
                        t_tiles[ft][:],
                        start=(ft == 0), stop=(ft == _FTG - 1))
                if fsg == 0:
                    nc.vector.tensor_copy(y_acc[dt][:], pdt[:])
                else:
                    nc.vector.tensor_add(y_acc[dt][:], y_acc[dt][:], pdt[:])

        prev = None  # (fsg, t_tiles, wd_tiles) of the previous f group
        for fsg in range(_NFSG):
            wg_t, wu_t = [], []
            if fsg > 0:
                for d in range(_ND):
                    g_t = wgp.tile([128, _FSG], f32r, tag=f"wg{d}")
                    nc.sync.dma_start(g_t[:], wg[fsg, d])
                    wg_t.append(g_t)
                    u_t = wup.tile([128, _FSG], f32r, tag=f"wu{d}")
                    nc.sync.dma_start(u_t[:], wu[fsg, d])
                    wu_t.append(u_t)

            def g_slice(d, ft):
                if fsg == 0:
                    q, r = divmod(ft * 128, qw)
                    return w0[("g", d, q)][:, r:r + 128]
                return wg_t[d][:, ft * 128:(ft + 1) * 128]

            def u_slice(d, ft):
                if fsg == 0:
                    q, r = divmod(ft * 128, qw)
                    return w0[("u", d, q)][:, r:r + 128]
                return wu_t[d][:, ft * 128:(ft + 1) * 128]

            t_tiles = []
            wd_tiles = []
            for ft in range(_FTG):
                fc = fsg * _FTG + ft
                wd_t = wdp.tile([128, D_MODEL], f32r, tag=f"wd{ft}")
                nc.sync.dma_start(wd_t[:], wd[fc * 128:(fc + 1) * 128, :])
                wd_tiles.append(wd_t)
                psg = pg.tile([128, W], f32)
                for d in range(_ND):
                    nc.tensor.matmul(
                        psg[:],
                        g_slice(d, ft),
                        xts[d][:],
                        start=(d == 0), stop=(d == _ND - 1))
                psu = pu.tile([128, W], f32)
                for d in range(_ND):
                    nc.tensor.matmul(
                        psu[:],
                        u_slice(d, ft),
                        xts[d][:],
                        start=(d == 0), stop=(d == _ND - 1))
                g_act = gap.tile([128, W], f32, tag="gact")
                nc.scalar.activation(g_act[:], psg[:],
                                     mybir.ActivationFunctionType.Silu)
                t_t = tp.tile([128, W], f32r, tag=f"t{ft}")
                nc.vector.tensor_mul(t_t[:], g_act[:], psu[:])
                t_tiles.append(t_t)
                if prev is not None:
                    if _FTG == 8:
                        emit_down(prev[0], prev[1], prev[2], (ft,))
                    else:
                        emit_down(prev[0], prev[1], prev[2], (2 * ft, 2 * ft + 1))
            prev = (fsg, t_tiles, wd_tiles)
        emit_down(prev[0], prev[1], prev[2], range(_ND))

        for dt in range(_ND):
            nc.sync.dma_start(yt[dt * 128:(dt + 1) * 128, :], y_acc[dt][:])

    nc.compile()
    return nc


def _pack_gu(w):
    # [D, F] -> [NFSG, ND, 128, FSG] so each streamed tile is contiguous
    w = np.asarray(w).astype(_np_cdt())
    return np.ascontiguousarray(
        w.reshape(_ND, 128, _NFSG, _FSG).transpose(2, 0, 1, 3))


def kernel(x, expert_idx, w_gate, w_up, w_down):
    from concourse.bass_utils import run_bass_kernel_spmd

    x = np.asarray(x, dtype=np.float32)
    idx = np.asarray(expert_idx).astype(np.int64)
    B, S, D = x.shape
    T = B * S
    x_flat = np.ascontiguousarray(x.reshape(T, D))
    idx_flat = idx.reshape(T)

    tok_lists = [np.nonzero(idx_flat == e)[0] for e in range(N_EXPERTS)]
    cap = max(1, max(len(t) for t in tok_lists))
    # multiple of 8 for DMA alignment; >=256 keeps float32r at full rate
    W = max(256, -(-cap // 8) * 8)
    assert W <= 512, f"per-expert token count {cap} exceeds single-pass capacity"

    key = W
    if key not in _nc_cache:
        _nc_cache[key] = _build_nc(W)
    nc = _nc_cache[key]

    in_maps = []
    for e in range(N_EXPERTS):
        toks = tok_lists[e]
        xt_e = np.zeros((D, W), dtype=_np_cdt())
        xt_e[:, :len(toks)] = x_flat[toks].T.astype(_np_cdt())
        in_maps.append({
            "xt": xt_e,
            "wg": _pack_gu(w_gate[e]),
            "wu": _pack_gu(w_up[e]),
            "wd": np.ascontiguousarray(np.asarray(w_down[e]).astype(_np_cdt())),
        })

    res = run_bass_kernel_spmd(nc, in_maps, core_ids=list(range(N_CORES)))

    out_flat = np.zeros((T, D), dtype=np.float32)
    for e in range(N_EXPERTS):
        toks = tok_lists[e]
        out_flat[toks] = res.results[e]["yt"][:, :len(toks)].T
    return out_flat.reshape(B, S, D)


# revision 14
# speedup vs baseline: 1.1907x; 1.1907x over previous
"""Expert-parallel MoE SwiGLU kernel for 8 Trainium2 NeuronCores.

Strategy: expert parallelism with host-side dispatch/combine. Each of the
8 cores owns one expert's weights. The host routes tokens by expert_idx,
packs each expert's tokens as a transposed [D, W] panel (features on
partitions so no on-chip transposes are needed anywhere), and each core
runs a dense SwiGLU FFN:  yT = w_down.T-blocks @ (silu(wg.T@xT) * (wu.T@xT)).
Matmuls run as float32r (full-rate fp32) with fp32 PSUM accumulation.
"""

import numpy as np
from contextlib import ExitStack

D_MODEL = 1024
D_FF = 4096
N_EXPERTS = 8
N_CORES = 8

_ND = D_MODEL // 128  # 8 contraction chunks over d_model
_NF = D_FF // 128     # 32 f chunks

_nc_cache = {}

# compute dtype for matmul operands: "float32r" (safest), "float16", "bfloat16"
import os as _os
_CDT = _os.environ.get("MOE_KERNEL_DTYPE", "float16")

# f columns per gate/up weight streaming group: keep DMA lines at 2KB
_FSG = 512 if _CDT == "float32r" else 1024
_NFSG = D_FF // _FSG
_FTG = _FSG // 128    # f-tiles per group


def _np_cdt():
    if _CDT == "float16":
        return np.float16
    if _CDT == "bfloat16":
        import ml_dtypes
        return ml_dtypes.bfloat16
    return np.float32


def _build_nc(W: int):
    """Build + schedule the per-core Bass program for token capacity W."""
    import concourse.bacc as bacc
    import concourse.tile as tile
    from concourse import mybir

    f32 = mybir.dt.float32
    f32r = getattr(mybir.dt, _CDT)

    nc = bacc.Bacc("TRN2", target_bir_lowering=False, debug=False,
                   num_devices=N_CORES)
    xt = nc.dram_tensor("xt", [D_MODEL, W], f32r, kind="ExternalInput").ap()
    wg = nc.dram_tensor("wg", [_NFSG, _ND, 128, _FSG], f32r,
                        kind="ExternalInput").ap()
    wu = nc.dram_tensor("wu", [_NFSG, _ND, 128, _FSG], f32r,
                        kind="ExternalInput").ap()
    wd = nc.dram_tensor("wd", [D_FF, D_MODEL], f32r, kind="ExternalInput").ap()
    yt = nc.dram_tensor("yt", [D_MODEL, W], f32, kind="ExternalOutput").ap()

    with tile.TileContext(nc) as tc, ExitStack() as ctx:
        xpool = ctx.enter_context(tc.tile_pool(name="x", bufs=1))
        wgp = ctx.enter_context(tc.tile_pool(name="wgp", bufs=3))
        wup = ctx.enter_context(tc.tile_pool(name="wup", bufs=3))
        wdp = ctx.enter_context(tc.tile_pool(name="wdp", bufs=3))
        tp = ctx.enter_context(tc.tile_pool(name="tp", bufs=2))
        gap = ctx.enter_context(tc.tile_pool(name="gap", bufs=3))
        yp = ctx.enter_context(tc.tile_pool(name="yp", bufs=1))
        pg = ctx.enter_context(tc.tile_pool(name="pg", bufs=2, space="PSUM"))
        pu = ctx.enter_context(tc.tile_pool(name="pu", bufs=2, space="PSUM"))
        pd = ctx.enter_context(tc.tile_pool(name="pd", bufs=4, space="PSUM"))

        # Input activations, transposed: d_model on partitions.
        xts = []
        for d in range(_ND):
            x_t = xpool.tile([128, W], f32r, tag=f"x{d}")
            nc.sync.dma_start(x_t[:], xt[d * 128:(d + 1) * 128, :])
            xts.append(x_t)

        y_acc = [yp.tile([128, W], f32, tag=f"y{d}", name=f"y_acc{d}")
                 for d in range(_ND)]

        # Fused pipeline over f groups: gate/up matmuls + SwiGLU produce
        # short-lived t tiles; the down-projection of the PREVIOUS f group
        # is interleaved between this group's matmul bursts so the PE's
        # DMA-wait gaps are broken into sub-HAM-window slices. Weight DMA
        # issue is split across both HWDGE rings (sync + scalar engines).
        def emit_down(fsg, t_tiles, wd_tiles, dts):
            # y[dt] += wd[fgroup rows, dt cols].T @ t   for dt in dts
            for dt in dts:
                pdt = pd.tile([128, W], f32, tag="pd", name=f"pd_{fsg}_{dt}")
                for ft in range(_FTG):
                    nc.tensor.matmul(
                        pdt[:],
                        wd_tiles[ft][:, dt * 128:(dt + 1) * 128],
                        t_tiles[ft][:],
                        start=(ft == 0), stop=(ft == _FTG - 1))
                if fsg == 0:
                    nc.vector.tensor_copy(y_acc[dt][:], pdt[:])
                else:
                    nc.vector.tensor_add(y_acc[dt][:], y_acc[dt][:], pdt[:])

        prev = None  # (fsg, t_tiles, wd_tiles) of the previous f group
        for fsg in range(_NFSG):
            wg_t, wu_t = [], []
            if fsg == 0:
                # gate tiles first: the first matmul group needs all 8
                for d in range(_ND):
                    g_t = wgp.tile([128, _FSG], f32r, tag=f"wg{d}")
                    nc.sync.dma_start(g_t[:], wg[fsg, d])
                    wg_t.append(g_t)
                for d in range(_ND):
                    u_t = wup.tile([128, _FSG], f32r, tag=f"wu{d}")
                    nc.sync.dma_start(u_t[:], wu[fsg, d])
                    wu_t.append(u_t)
            else:
                for d in range(_ND):
                    g_t = wgp.tile([128, _FSG], f32r, tag=f"wg{d}")
                    nc.sync.dma_start(g_t[:], wg[fsg, d])
                    wg_t.append(g_t)
                    u_t = wup.tile([128, _FSG], f32r, tag=f"wu{d}")
                    nc.sync.dma_start(u_t[:], wu[fsg, d])
                    wu_t.append(u_t)

            def g_slice(d, ft):
                return wg_t[d][:, ft * 128:(ft + 1) * 128]

            def u_slice(d, ft):
                return wu_t[d][:, ft * 128:(ft + 1) * 128]

            t_tiles = []
            wd_tiles = []
            for ft in range(_FTG):
                fc = fsg * _FTG + ft
                wd_t = wdp.tile([128, D_MODEL], f32r, tag=f"wd{ft}")
                nc.sync.dma_start(wd_t[:], wd[fc * 128:(fc + 1) * 128, :])
                wd_tiles.append(wd_t)
                psg = pg.tile([128, W], f32)
                for d in range(_ND):
                    nc.tensor.matmul(
                        psg[:],
                        g_slice(d, ft),
                        xts[d][:],
                        start=(d == 0), stop=(d == _ND - 1))
                psu = pu.tile([128, W], f32)
                for d in range(_ND):
                    nc.tensor.matmul(
                        psu[:],
                        u_slice(d, ft),
                        xts[d][:],
                        start=(d == 0), stop=(d == _ND - 1))
                g_act = gap.tile([128, W], f32, tag="gact")
                nc.scalar.activation(g_act[:], psg[:],
                                     mybir.ActivationFunctionType.Silu)
                t_t = tp.tile([128, W], f32r, tag=f"t{ft}")
                nc.vector.tensor_mul(t_t[:], g_act[:], psu[:])
                t_tiles.append(t_t)
                if prev is not None:
                    if _FTG == 8:
                        emit_down(prev[0], prev[1], prev[2], (ft,))
                    else:
                        emit_down(prev[0], prev[1], prev[2], (2 * ft, 2 * ft + 1))
            prev = (fsg, t_tiles, wd_tiles)
        emit_down(prev[0], prev[1], prev[2], range(_ND))

        for dt in range(_ND):
            nc.sync.dma_start(yt[dt * 128:(dt + 1) * 128, :], y_acc[dt][:])

    nc.compile()
    return nc


def _pack_gu(w):
    # [D, F] -> [NFSG, ND, 128, FSG] so each streamed tile is contiguous
    w = np.asarray(w).astype(_np_cdt())
    return np.ascontiguousarray(
        w.reshape(_ND, 128, _NFSG, _FSG).transpose(2, 0, 1, 3))


def kernel(x, expert_idx, w_gate, w_up, w_down):
    from concourse.bass_utils import run_bass_kernel_spmd

    x = np.asarray(x, dtype=np.float32)
    idx = np.asarray(expert_idx).astype(np.int64)
    B, S, D = x.shape
    T = B * S
    x_flat = np.ascontiguousarray(x.reshape(T, D))
    idx_flat = idx.reshape(T)

    tok_lists = [np.nonzero(idx_flat == e)[0] for e in range(N_EXPERTS)]
    cap = max(1, max(len(t) for t in tok_lists))
    # multiple of 8 for DMA alignment; >=256 keeps float32r at full rate
    W = max(256, -(-cap // 8) * 8)
    assert W <= 512, f"per-expert token count {cap} exceeds single-pass capacity"

    key = W
    if key not in _nc_cache:
        _nc_cache[key] = _build_nc(W)
    nc = _nc_cache[key]

    in_maps = []
    for e in range(N_EXPERTS):
        toks = tok_lists[e]
        xt_e = np.zeros((D, W), dtype=_np_cdt())
        xt_e[:, :len(toks)] = x_flat[toks].T.astype(_np_cdt())
        in_maps.append({
            "xt": xt_e,
            "wg": _pack_gu(w_gate[e]),
            "wu": _pack_gu(w_up[e]),
            "wd": np.ascontiguousarray(np.asarray(w_down[e]).astype(_np_cdt())),
        })

    res = run_bass_kernel_spmd(nc, in_maps, core_ids=list(range(N_CORES)))

    out_flat = np.zeros((T, D), dtype=np.float32)
    for e in range(N_EXPERTS):
        toks = tok_lists[e]
        out_flat[toks] = res.results[e]["yt"][:, :len(toks)].T
    return out_flat.reshape(B, S, D)
